# revision 30
# baseline (speedup 1.0000x reference)
"""Trainium2 Bass kernel for nn_MultiHeadAttention_54580444397743.

Head-sharded tensor parallel over 8 NeuronCores: 2 heads/core, both batches.

Precision scheme (relies on fp32r PE operand rounding ~12 mantissa bits;
validated in numpy sim at ~4e-3 rel err vs 2e-2 gate):
  - projections: single fp32r MM per c-tile (W r12'd on host, x raw)
  - K split on-chip into hi/lo (12-bit chunks); kc = [Khi(64); Klo(63); -1]
  - scores^T in ONE K=128 MM: kc^T @ [Q; Q(63); m] = S - m (max folded in)
  - natural-orientation pass reuses the same operands for the row max
  - P^T = exp(S^T - m); PV via [V|1] augmented matmul gives out + denominator

Schedule: the two batches run phase-shifted (proj/nat/attn) with interleaved
instruction emission so the PE never idles (HAM stays at 2.4 GHz).
"""

import numpy as np

import concourse.bass as bass
import concourse.mybir as mybir
from concourse import bacc, bass_utils
from concourse.tile import TileContext
from concourse.masks import make_identity

B, T, C = 2, 2048, 1024
H, D = 16, 64
NCORES = 8
TB = 512                   # t-block width (PSUM bank)
NTB = T // TB              # 4 t-blocks per batch
NCT = C // 128             # 8 c-tiles
F32 = mybir.dt.float32
F32R = mybir.dt.float32r
NEG = -1.0e9
AX = mybir.AxisListType.X
COPY = mybir.ActivationFunctionType.Copy
EXP = mybir.ActivationFunctionType.Exp
BF16 = mybir.dt.bfloat16

GP_REDUCE = False   # Pool engine cannot do free-axis reduces
GP_YEVAC = False    # GPSIMD cannot access PSUM


def _r12(x):
    m, e = np.frexp(np.asarray(x, np.float64))
    return np.ldexp(np.round(m * 4096.0) / 4096.0, e).astype(np.float32)


def _ceildiv(a, b):
    return -(-a // b)


def _interleave(gens, weights=None):
    gens = [g for g in gens if g is not None]
    if weights is None:
        weights = [1] * len(gens)
    weights = list(weights[:len(gens)])
    while gens:
        alive, walive = [], []
        for g, w in zip(gens, weights):
            ok = True
            for _ in range(w):
                try:
                    next(g)
                except StopIteration:
                    ok = False
                    break
            if ok:
                alive.append(g)
                walive.append(w)
        gens, weights = alive, walive


def build_nc():
    nc = bacc.Bacc(target_bir_lowering=False, debug=False)

    x = nc.dram_tensor("x", [B, C, T], F32R, kind="ExternalInput")
    wq = nc.dram_tensor("wq", [C, 128], F32R, kind="ExternalInput")
    wk = nc.dram_tensor("wk", [C, 128], F32R, kind="ExternalInput")
    wv = nc.dram_tensor("wv", [C, 128], F32R, kind="ExternalInput")
    wp = nc.dram_tensor("wp", [128, C], F32R, kind="ExternalInput")
    mask_nat = nc.dram_tensor("mask_nat", [128, 128], F32, kind="ExternalInput")
    mask_t = nc.dram_tensor("mask_t", [128, 128], F32, kind="ExternalInput")
    y = nc.dram_tensor("y", [B, T, C], BF16, kind="ExternalOutput")

    with TileContext(nc) as tc:
        with (
            tc.tile_pool(name="sbW", bufs=1) as sbW,
            tc.tile_pool(name="sbBig", bufs=1) as sbBig,
            tc.tile_pool(name="sbX", bufs=16) as sbX,
            tc.tile_pool(name="sbS", bufs=2) as sbS,
            tc.tile_pool(name="sbP", bufs=4) as sbP,
            tc.tile_pool(name="sbY", bufs=2) as sbY,
            tc.tile_pool(name="psX", bufs=3, space="PSUM") as psX,
            tc.tile_pool(name="psS", bufs=3, space="PSUM") as psS,
            tc.tile_pool(name="psO", bufs=1, space="PSUM") as psO,
        ):
            xtiles = {}

            def issue_x(b, tb, eng=None):
                eng = eng or nc.sync
                ts = slice(TB * tb, TB * (tb + 1))
                tiles = []
                for j in range(NCT):
                    cs = slice(128 * j, 128 * (j + 1))
                    tx = sbX.tile([128, TB], F32R, name=f"x_{b}_{tb}_{j}", tag="x")
                    eng.dma_start(out=tx[:, :], in_=x[b, cs, ts])
                    tiles.append(tx)
                xtiles[(b, tb)] = tiles

            # ---------------- constants / weights ----------------
            t_wq = sbW.tile([128, C], F32R, name="t_wq")
            t_wk = sbW.tile([128, C], F32R, name="t_wk")
            t_wv = sbW.tile([128, C], F32R, name="t_wv")
            t_wp = sbW.tile([128, C], F32R, name="t_wp")
            issue_x(0, 0, eng=nc.gpsimd)
            for j in range(NCT):
                cs = slice(128 * j, 128 * (j + 1))
                nc.sync.dma_start(out=t_wq[:, cs], in_=wq[cs, :])
                nc.scalar.dma_start(out=t_wk[:, cs], in_=wk[cs, :])
                nc.gpsimd.dma_start(out=t_wv[:, cs], in_=wv[cs, :])
            nc.scalar.dma_start(out=t_wp[:, :], in_=wp[:, :])
            t_mnat = sbW.tile([128, 128], F32, name="t_mnat")
            t_mt = sbW.tile([128, 128], F32, name="t_mt")
            nc.scalar.dma_start(out=t_mnat[:, :], in_=mask_nat[:, :])
            nc.scalar.dma_start(out=t_mt[:, :], in_=mask_t[:, :])

            # warm the PE clock gate while the first x/w DMAs are in flight;
            # input is a memset tile so the chain is ready almost immediately
            t_wupin = sbW.tile([128, TB], F32R, name="t_wupin")
            nc.vector.memset(t_wupin[:, :].bitcast(F32), 0.5)
            for w in range(20):
                wup = psX.tile([128, TB], F32, name=f"wup_{w}", tag="x")
                nc.tensor.matmul(wup[:, :], t_wupin[:, 0:128], t_wupin[:, :],
                                 start=True, stop=True)
            t_idf = sbW.tile([128, 128], F32, name="t_idf")
            make_identity(nc, t_idf)
            t_id = sbW.tile([128, 128], F32R, name="t_id")
            nc.vector.tensor_copy(t_id[:, :], t_idf[:, :])
            # rows 64 of ones65a/b select head0/head1 cols for the recip bcast
            ones65a = sbW.tile([65, 128], F32R, name="ones65a")
            ones65b = sbW.tile([65, 128], F32R, name="ones65b")
            nc.vector.memset(ones65a[64:65, :].bitcast(F32), 0.0)
            nc.vector.memset(ones65b[64:65, :].bitcast(F32), 0.0)
            nc.vector.memset(ones65a[64:65, 0:64].bitcast(F32), 1.0)
            nc.vector.memset(ones65b[64:65, 64:128].bitcast(F32), 1.0)

            # ---------------- per-batch persistent tensors ----------------
            kc0, kc1, q20, q21, vaug = {}, {}, {}, {}, {}
            for b in range(B):
                kc0[b] = sbBig.tile([128, T], F32R, name=f"kc0_{b}", tag=f"kc0_{b}")
                kc1[b] = sbBig.tile([128, T], F32R, name=f"kc1_{b}", tag=f"kc1_{b}")
                q20[b] = sbBig.tile([128, T], F32R, name=f"q20_{b}", tag=f"q20_{b}")
                q21[b] = sbBig.tile([128, T], F32R, name=f"q21_{b}", tag=f"q21_{b}")
                vaug[b] = [
                    sbBig.tile([128, 130], F32R, name=f"vaug_{b}_{st}", tag=f"vaug_{b}_{st}")
                    for st in range(T // 128)
                ]

            # ---------------- phase generators ----------------
            def gen_proj(b, tb):
                if (b, tb) == (0, 0):
                    issue_x(1, 0)
                ts = slice(TB * tb, TB * (tb + 1))
                tiles = xtiles.pop((b, tb))
                # ---- Q ----
                qp = psX.tile([128, TB], F32, name=f"qp_{b}_{tb}", tag="x")
                for j in range(NCT):
                    cs = slice(128 * j, 128 * (j + 1))
                    nc.tensor.matmul(qp[:, :], t_wq[:, cs], tiles[j][:, :],
                                     start=(j == 0), stop=(j == NCT - 1))
                    if j % 2 == 1:
                        yield
                # evac: q20 rows 0:64 direct; q21 via staged shift; dup rows 64:127
                nc.scalar.activation(q20[b][0:64, ts], qp[0:64, :], COPY)
                qstg = sbS.tile([128, TB], F32R, name=f"qstg_{b}_{tb}", tag="qstg", bufs=1)
                nc.scalar.activation(qstg[64:128, :], qp[64:128, :], COPY)
                nc.sync.dma_start(out=q20[b][64:96, ts], in_=q20[b][0:32, ts])
                nc.sync.dma_start(out=q21[b][0:64, ts], in_=qstg[64:128, :])
                nc.sync.dma_start(out=q21[b][64:96, ts], in_=qstg[64:96, :])
                yield
                # ---- K ----
                kp = psX.tile([128, TB], F32, name=f"kp_{b}_{tb}", tag="x")
                for j in range(NCT):
                    cs = slice(128 * j, 128 * (j + 1))
                    nc.tensor.matmul(kp[:, :], t_wk[:, cs], tiles[j][:, :],
                                     start=(j == 0), stop=(j == NCT - 1))
                    if j % 2 == 1:
                        yield
                nc.vector.memset(kc0[b][96:97, ts].bitcast(F32), -1.0)
                nc.vector.memset(kc1[b][96:97, ts].bitcast(F32), -1.0)
                khi = sbS.tile([128, TB], F32R, name=f"khi_{b}_{tb}", tag="khi", bufs=1)
                klo = sbS.tile([128, TB], F32R, name=f"klo_{b}_{tb}", tag="klo", bufs=1)
                nc.vector.tensor_copy(khi[:, :], kp[:, :])
                nc.vector.tensor_sub(klo[:, :], kp[:, :], khi[:, :].bitcast(F32))
                nc.gpsimd.tensor_copy(kc0[b][0:64, ts], khi[0:64, :])
                nc.sync.dma_start(out=kc1[b][0:64, ts], in_=khi[64:128, :])
                nc.sync.dma_start(out=kc0[b][64:96, ts], in_=klo[0:32, :])
                nc.sync.dma_start(out=kc1[b][64:96, ts], in_=klo[64:96, :])
                yield
                # ---- V ----
                vp = psX.tile([128, TB], F32, name=f"vp_{b}_{tb}", tag="x")
                for j in range(NCT):
                    cs = slice(128 * j, 128 * (j + 1))
                    nc.tensor.matmul(vp[:, :], t_wv[:, cs], tiles[j][:, :],
                                     start=(j == 0), stop=(j == NCT - 1))
                    if j % 2 == 1:
                        yield
                vtr = sbS.tile([128, TB], F32R, name=f"vtr_{b}_{tb}", tag="vtr", bufs=1)
                nc.vector.tensor_copy(vtr[:, :], vp[:, :])
                yield
                for i in range(4):
                    st = 4 * tb + i
                    tvp = psX.tile([128, 128], F32R, name=f"tvp_{b}_{st}", tag="x")
                    nc.tensor.transpose(tvp[:, :], vtr[:, 128 * i:128 * (i + 1)], t_id[:, :])
                    va = vaug[b][st]
                    nc.gpsimd.memset(va[:, 64:65].bitcast(F32), 1.0)
                    nc.gpsimd.memset(va[:, 129:130].bitcast(F32), 1.0)
                    nc.scalar.activation(va[:, 0:64], tvp[:, 0:64], COPY)
                    nc.scalar.activation(va[:, 65:129], tvp[:, 64:128], COPY)
                    yield

            def gen_nat(b, tb):
                ts = slice(TB * tb, TB * (tb + 1))
                pend = []
                yield  # kc/q2 assembly DMAs land while the partner runs
                for i in range(4):
                    gt = 4 * tb + i
                    tts = slice(128 * gt, 128 * (gt + 1))
                    cols = 128 * (gt + 1)
                    nb = _ceildiv(cols, TB)
                    msc0 = sbS.tile([128, 128], F32R, name=f"msc0_{b}_{gt}", tag="msc0")
                    msc1 = sbS.tile([128, 128], F32R, name=f"msc1_{b}_{gt}", tag="msc1")
                    for j in range(nb):
                        w = min(TB, cols - TB * j)
                        ss = slice(TB * j, TB * j + w)
                        np0 = psX.tile([128, TB], F32, name=f"np0_{b}_{gt}_{j}", tag="x")
                        np1 = psX.tile([128, TB], F32, name=f"np1_{b}_{gt}_{j}", tag="x")
                        nc.tensor.matmul(np0[:, :w], q20[b][0:96, tts], kc0[b][0:96, ss],
                                         start=True, stop=True)
                        nc.tensor.matmul(np1[:, :w], q21[b][0:96, tts], kc1[b][0:96, ss],
                                         start=True, stop=True)
                        yield
                        if j == nb - 1:
                            dsl = slice(w - 128, w)
                            nc.vector.tensor_add(np0[:, dsl], np0[:, dsl], t_mnat[:, :])
                            nc.vector.tensor_add(np1[:, dsl], np1[:, dsl], t_mnat[:, :])
                        nc.vector.reduce_max(msc0[:, j:j + 1], np0[:, :w], axis=AX)
                        if GP_REDUCE:
                            nc.gpsimd.reduce_max(msc1[:, j:j + 1], np1[:, :w], axis=AX)
                        else:
                            nc.vector.reduce_max(msc1[:, j:j + 1], np1[:, :w], axis=AX)
                    nc.vector.reduce_max(msc0[:, 96:97], msc0[:, 0:nb], axis=AX)
                    if GP_REDUCE:
                        nc.gpsimd.reduce_max(msc1[:, 127:128], msc1[:, 0:nb], axis=AX)
                    else:
                        nc.vector.reduce_max(msc1[:, 96:97], msc1[:, 0:nb], axis=AX)
                    pend.append((i, msc0, msc1))
                    # delay the max transposes one t-tile so the reduces drain
                    if len(pend) >= 2:
                        yield from _emit_mtr(b, tb, pend.pop(0))
                while pend:
                    yield from _emit_mtr(b, tb, pend.pop(0))
                # m rows -> q2 row 127 (partition-aligned copies from psum)
                tps = mtrs.pop((b, tb))
                for k, (i, tp0, tp1) in enumerate(tps):
                    isl = slice(TB * tb + 128 * i, TB * tb + 128 * (i + 1))
                    nc.scalar.activation(q20[b][96:97, isl], tp0[96:97, :], COPY)
                    nc.scalar.activation(q21[b][96:97, isl], tp1[96:97, :], COPY)
                yield

            mtrs = {}

            def _emit_mtr(b, tb, item):
                i, msc0, msc1 = item
                tp0 = psX.tile([128, 128], F32R, name=f"tp0_{b}_{tb}_{i}", tag="x")
                tp1 = psX.tile([128, 128], F32R, name=f"tp1_{b}_{tb}_{i}", tag="x")
                nc.tensor.transpose(tp0[:, :], msc0[:, :], t_id[:, :])
                nc.tensor.transpose(tp1[:, :], msc1[:, :], t_id[:, :])
                mtrs.setdefault((b, tb), []).append((i, tp0, tp1))
                yield

            onorms = {}

            def gen_att(b, tb):
                if tb + 1 < NTB:
                    issue_x(b, tb + 1)
                for _ in range(3):
                    yield  # let the partner phase queue PE work ahead of sp0
                t0 = TB * tb
                ts = slice(t0, t0 + TB)
                ov0 = psO.tile([128, TB], F32, name=f"ov0_{b}_{tb}", tag="ov0")
                ov1 = psO.tile([128, TB], F32, name=f"ov1_{b}_{tb}", tag="ov1")
                last = 4 * (tb + 1) - 1
                for st in range(4 * (tb + 1)):
                    diag = st >= 4 * tb
                    coff = 128 * st - t0 if diag else 0
                    sl = slice(coff, TB)
                    tsl = slice(t0 + coff, t0 + TB)
                    sts = slice(128 * st, 128 * (st + 1))
                    sp0 = psS.tile([128, TB], F32, name=f"sp0_{b}_{tb}_{st}", tag="sp")
                    sp1 = psS.tile([128, TB], F32, name=f"sp1_{b}_{tb}_{st}", tag="sp")
                    pt0 = sbP.tile([128, TB], F32R, name=f"pt0_{b}_{tb}_{st}", tag="pt0")
                    pt1 = sbP.tile([128, TB], F32R, name=f"pt1_{b}_{tb}_{st}", tag="pt1")
                    nc.tensor.matmul(sp0[:, sl], kc0[b][0:97, sts], q20[b][0:97, tsl],
                                     start=True, stop=True)
                    if diag:
                        dsl = slice(coff, coff + 128)
                        nc.vector.tensor_add(sp0[:, dsl], sp0[:, dsl], t_mt[:, :])
                    nc.scalar.activation(pt0[:, sl], sp0[:, sl], EXP)
                    nc.tensor.matmul(sp1[:, sl], kc1[b][0:97, sts], q21[b][0:97, tsl],
                                     start=True, stop=True)
                    if diag:
                        dsl = slice(coff, coff + 128)
                        nc.vector.tensor_add(sp1[:, dsl], sp1[:, dsl], t_mt[:, :])
                    nc.scalar.activation(pt1[:, sl], sp1[:, sl], EXP)
                    yield
                    nc.tensor.matmul(ov0[0:65, sl], vaug[b][st][:, 0:65], pt0[:, sl],
                                     start=(st == 0), stop=(st == last))
                    nc.tensor.matmul(ov1[0:65, sl], vaug[b][st][:, 65:130], pt1[:, sl],
                                     start=(st == 0), stop=(st == last))
                    yield
                # ---- normalize ----
                # broadcast the denominators to [128,TB] via K=1 matmuls, then a
                # single 128-lane reciprocal (a [1,TB] reciprocal is ~6x slower)
                rcA = sbS.tile([65, TB], F32R, name=f"rcA_{b}_{tb}", tag="rcA", bufs=1)
                rcB = sbS.tile([65, TB], F32R, name=f"rcB_{b}_{tb}", tag="rcB", bufs=1)
                nc.vector.tensor_copy(rcA[64:65, :], ov0[64:65, :])
                nc.vector.tensor_copy(rcB[64:65, :], ov1[64:65, :])
                bc = psS.tile([128, TB], F32, name=f"bc_{b}_{tb}", tag="sp")
                nc.tensor.matmul(bc[:, :], ones65a[64:65, :], rcA[64:65, :],
                                 start=True, stop=False)
                nc.tensor.matmul(bc[:, :], ones65b[64:65, :], rcB[64:65, :],
                                 start=False, stop=True)
                rb_sb = sbS.tile([128, TB], F32, name=f"rb_{b}_{tb}", tag="rb", bufs=1)
                nc.vector.reciprocal_approx_fast(rb_sb[:, :], bc[:, :])
                ost = sbS.tile([128, TB], F32, name=f"ost_{b}_{tb}", tag="ost")
                stg1 = sbS.tile([64, TB], F32, name=f"stg1_{b}_{tb}", tag="stg1")
                nc.scalar.activation(ost[0:64, :], ov0[0:64, :], COPY)
                nc.scalar.activation(stg1[:, :], ov1[0:64, :], COPY)
                nc.sync.dma_start(out=ost[64:128, :], in_=stg1[:, :])
                onorm = sbS.tile([128, TB], F32R, name=f"onorm_{b}_{tb}", tag="onorm")
                nc.vector.tensor_mul(onorm[:, :], ost[:, :], rb_sb[:, :])
                yield
                onorms[(b, tb)] = onorm

            def gen_outproj(b, tb):
                onorm = onorms.pop((b, tb))
                # ---- output projection ----
                for tt in range(4):
                    gtt = 4 * tb + tt
                    tloc = slice(128 * tt, 128 * (tt + 1))
                    ysb = sbY.tile([128, C], BF16, name=f"ysb_{b}_{gtt}", tag="ysb")
                    for e in range(2):
                        es = slice(512 * e, 512 * (e + 1))
                        yp = psX.tile([128, 512], F32, name=f"yp_{b}_{gtt}_{e}", tag="x")
                        nc.tensor.matmul(yp[:, :], onorm[:, tloc], t_wp[:, es],
                                         start=True, stop=True)
                        if e == 0:
                            nc.scalar.activation(ysb[:, es], yp[:, :], COPY)
                        else:
                            nc.vector.tensor_copy(ysb[:, es], yp[:, :])
                        yield
                    # half-tile rows keep 2KB/partition lines but start draining
                    # earlier and shorten the final DMA tail
                    for h in range(2):
                        hs = slice(64 * h, 64 * (h + 1))
                        nc.gpsimd.dma_start(
                            out=y[b, slice(128 * gtt + 64 * h, 128 * gtt + 64 * (h + 1)), :],
                            in_=ysb[hs, :])
                        yield

            # ---------------- phase-shifted schedule ----------------
            def make_gen(kind, b, tb):
                if kind == "P":
                    return gen_proj(b, tb)
                if kind == "N":
                    return gen_nat(b, tb)
                return gen_att(b, tb)

            stream0 = [(k, 0, tb) for tb in range(NTB) for k in ("P", "N", "S")]
            stream1 = [(k, 1, tb) for tb in range(NTB) for k in ("P", "N", "S")]
            WKIND = {"P": 2, "N": 1, "S": 3}
            nslots = len(stream0) + 2
            prev_s = []
            for i in range(nslots):
                gens, weights = [], []
                cur_s = []
                if i < len(stream0):
                    k, bb, tb = stream0[i]
                    gens.append(make_gen(k, bb, tb))
                    weights.append(WKIND[k])
                    if k == "S":
                        cur_s.append((bb, tb))
                if 1 <= i <= len(stream1):
                    k1, bb1, tb1 = stream1[i - 1]
                    gens.append(make_gen(k1, bb1, tb1))
                    weights.append(WKIND[k1])
                    if k1 == "S":
                        cur_s.append((bb1, tb1))
                for (bb, tb) in prev_s:
                    gens.append(gen_outproj(bb, tb))
                    weights.append(1)
                prev_s = cur_s
                _interleave(gens, weights)

    nc.compile()
    return nc


_NC_CACHE = {}


def _get_nc():
    if "nc" not in _NC_CACHE:
        _NC_CACHE["nc"] = build_nc()
    return _NC_CACHE["nc"]


def prepare_in_maps(ix, Wq, Wk, Wv, Wp):
    ix = np.asarray(ix, np.float32)
    Wq = np.asarray(Wq, np.float32)
    Wk = np.asarray(Wk, np.float32)
    Wv = np.asarray(Wv, np.float32)
    Wp = np.asarray(Wp, np.float32)

    xt = np.ascontiguousarray(ix.transpose(0, 2, 1))  # [B, C, T]
    mask = np.where(np.arange(128)[:, None] >= np.arange(128)[None, :], 0.0, NEG).astype(np.float32)

    in_maps = []
    for core in range(NCORES):
        h0, h1 = 2 * core, 2 * core + 1
        in_maps.append({
            "x": xt,
            "wq": _r12(np.concatenate([Wq[h0], Wq[h1]], axis=1) * 32.0),  # carries sqrt(C)
            "wk": _r12(np.concatenate([Wk[h0], Wk[h1]], axis=1)),
            "wv": _r12(np.concatenate([Wv[h0], Wv[h1]], axis=1)),
            "wp": _r12(Wp[D * h0:D * (h1 + 1), :]),
            "mask_nat": mask,
            "mask_t": np.ascontiguousarray(mask.T),
        })
    return in_maps


def kernel(ix, Wq, Wk, Wv, Wp, bp):
    in_maps = prepare_in_maps(ix, Wq, Wk, Wv, Wp)
    bp = np.asarray(bp, np.float32)
    nc = _get_nc()
    res = bass_utils.run_bass_kernel_spmd(nc, in_maps, list(range(NCORES)))
    out = np.zeros((B, T, C), np.float64)
    for r in res.results:
        out += np.asarray(r["y"], np.float32)
    out += bp
    return out.astype(np.float32)


# revision 31
# speedup vs baseline: 1.0220x; 1.0220x over previous
"""Trainium2 Bass kernel for nn_MultiHeadAttention_54580444397743.

Head-sharded tensor parallel over 8 NeuronCores: 2 heads/core, both batches.

Precision scheme (relies on fp32r PE operand rounding ~12 mantissa bits;
validated in numpy sim at ~4e-3 rel err vs 2e-2 gate):
  - projections: single fp32r MM per c-tile (W r12'd on host, x raw)
  - K split on-chip into hi/lo (12-bit chunks); kc = [Khi(64); Klo(63); -1]
  - scores^T in ONE K=128 MM: kc^T @ [Q; Q(63); m] = S - m (max folded in)
  - natural-orientation pass reuses the same operands for the row max
  - P^T = exp(S^T - m); PV via [V|1] augmented matmul gives out + denominator

Schedule: the two batches run phase-shifted (proj/nat/attn) with interleaved
instruction emission so the PE never idles (HAM stays at 2.4 GHz).
"""

import numpy as np

import concourse.bass as bass
import concourse.mybir as mybir
from concourse import bacc, bass_utils
from concourse.tile import TileContext
from concourse.masks import make_identity

B, T, C = 2, 2048, 1024
H, D = 16, 64
NCORES = 8
TB = 512                   # t-block width (PSUM bank)
NTB = T // TB              # 4 t-blocks per batch
NCT = C // 128             # 8 c-tiles
F32 = mybir.dt.float32
F32R = mybir.dt.float32r
NEG = -1.0e9
AX = mybir.AxisListType.X
COPY = mybir.ActivationFunctionType.Copy
EXP = mybir.ActivationFunctionType.Exp
BF16 = mybir.dt.bfloat16

GP_REDUCE = False   # Pool engine cannot do free-axis reduces
GP_YEVAC = False    # GPSIMD cannot access PSUM


def _r12(x):
    m, e = np.frexp(np.asarray(x, np.float64))
    return np.ldexp(np.round(m * 4096.0) / 4096.0, e).astype(np.float32)


def _ceildiv(a, b):
    return -(-a // b)


def _interleave(gens, weights=None):
    gens = [g for g in gens if g is not None]
    if weights is None:
        weights = [1] * len(gens)
    weights = list(weights[:len(gens)])
    while gens:
        alive, walive = [], []
        for g, w in zip(gens, weights):
            ok = True
            for _ in range(w):
                try:
                    next(g)
                except StopIteration:
                    ok = False
                    break
            if ok:
                alive.append(g)
                walive.append(w)
        gens, weights = alive, walive


def build_nc():
    nc = bacc.Bacc(target_bir_lowering=False, debug=False)

    x = nc.dram_tensor("x", [B, C, T], F32R, kind="ExternalInput")
    wq = nc.dram_tensor("wq", [C, 128], F32R, kind="ExternalInput")
    wk = nc.dram_tensor("wk", [C, 128], F32R, kind="ExternalInput")
    wv = nc.dram_tensor("wv", [C, 128], F32R, kind="ExternalInput")
    wp = nc.dram_tensor("wp", [128, C], F32R, kind="ExternalInput")
    mask_nat = nc.dram_tensor("mask_nat", [128, 128], F32, kind="ExternalInput")
    mask_t = nc.dram_tensor("mask_t", [128, 128], F32, kind="ExternalInput")
    y = nc.dram_tensor("y", [B, T, C], BF16, kind="ExternalOutput")

    with TileContext(nc) as tc:
        with (
            tc.tile_pool(name="sbW", bufs=1) as sbW,
            tc.tile_pool(name="sbBig", bufs=1) as sbBig,
            tc.tile_pool(name="sbX", bufs=16) as sbX,
            tc.tile_pool(name="sbS", bufs=2) as sbS,
            tc.tile_pool(name="sbP", bufs=4) as sbP,
            tc.tile_pool(name="sbY", bufs=2) as sbY,
            tc.tile_pool(name="psX", bufs=3, space="PSUM") as psX,
            tc.tile_pool(name="psS", bufs=3, space="PSUM") as psS,
            tc.tile_pool(name="psO", bufs=1, space="PSUM") as psO,
        ):
            xtiles = {}

            def issue_x(b, tb, eng=None):
                eng = eng or nc.sync
                ts = slice(TB * tb, TB * (tb + 1))
                tiles = []
                for j in range(NCT):
                    cs = slice(128 * j, 128 * (j + 1))
                    tx = sbX.tile([128, TB], F32R, name=f"x_{b}_{tb}_{j}", tag="x")
                    eng.dma_start(out=tx[:, :], in_=x[b, cs, ts])
                    tiles.append(tx)
                xtiles[(b, tb)] = tiles

            # ---------------- constants / weights ----------------
            t_wq = sbW.tile([128, C], F32R, name="t_wq")
            t_wk = sbW.tile([128, C], F32R, name="t_wk")
            t_wv = sbW.tile([128, C], F32R, name="t_wv")
            t_wp = sbW.tile([128, C], F32R, name="t_wp")
            issue_x(0, 0, eng=nc.gpsimd)
            for j in range(NCT):
                cs = slice(128 * j, 128 * (j + 1))
                nc.sync.dma_start(out=t_wq[:, cs], in_=wq[cs, :])
                nc.scalar.dma_start(out=t_wk[:, cs], in_=wk[cs, :])
                nc.gpsimd.dma_start(out=t_wv[:, cs], in_=wv[cs, :])
            nc.scalar.dma_start(out=t_wp[:, :], in_=wp[:, :])
            t_mnat = sbW.tile([128, 128], F32, name="t_mnat")
            t_mt = sbW.tile([128, 128], F32, name="t_mt")
            nc.scalar.dma_start(out=t_mnat[:, :], in_=mask_nat[:, :])
            nc.scalar.dma_start(out=t_mt[:, :], in_=mask_t[:, :])

            # warm the PE clock gate while the first x/w DMAs are in flight;
            # input is a memset tile so the chain is ready almost immediately
            t_wupin = sbW.tile([128, TB], F32R, name="t_wupin")
            nc.vector.memset(t_wupin[:, :].bitcast(F32), 0.5)
            for w in range(20):
                wup = psX.tile([128, TB], F32, name=f"wup_{w}", tag="x")
                nc.tensor.matmul(wup[:, :], t_wupin[:, 0:128], t_wupin[:, :],
                                 start=True, stop=True)
            t_idf = sbW.tile([128, 128], F32, name="t_idf")
            make_identity(nc, t_idf)
            t_id = sbW.tile([128, 128], F32R, name="t_id")
            nc.vector.tensor_copy(t_id[:, :], t_idf[:, :])
            # rows 64 of ones65a/b select head0/head1 cols for the recip bcast
            ones65a = sbW.tile([65, 128], F32R, name="ones65a")
            ones65b = sbW.tile([65, 128], F32R, name="ones65b")
            nc.vector.memset(ones65a[64:65, :].bitcast(F32), 0.0)
            nc.vector.memset(ones65b[64:65, :].bitcast(F32), 0.0)
            nc.vector.memset(ones65a[64:65, 0:64].bitcast(F32), 1.0)
            nc.vector.memset(ones65b[64:65, 64:128].bitcast(F32), 1.0)

            # ---------------- per-batch persistent tensors ----------------
            kc0, kc1, q20, q21, vaug = {}, {}, {}, {}, {}
            for b in range(B):
                kc0[b] = sbBig.tile([128, T], F32R, name=f"kc0_{b}", tag=f"kc0_{b}")
                kc1[b] = sbBig.tile([128, T], F32R, name=f"kc1_{b}", tag=f"kc1_{b}")
                q20[b] = sbBig.tile([128, T], F32R, name=f"q20_{b}", tag=f"q20_{b}")
                q21[b] = sbBig.tile([128, T], F32R, name=f"q21_{b}", tag=f"q21_{b}")
                vaug[b] = [
                    sbBig.tile([128, 130], F32R, name=f"vaug_{b}_{st}", tag=f"vaug_{b}_{st}")
                    for st in range(T // 128)
                ]

            # ---------------- phase generators ----------------
            def gen_proj(b, tb):
                if (b, tb) == (0, 0):
                    issue_x(1, 0)
                ts = slice(TB * tb, TB * (tb + 1))
                tiles = xtiles.pop((b, tb))
                # ---- Q ----
                qp = psX.tile([128, TB], F32, name=f"qp_{b}_{tb}", tag="x")
                for j in range(NCT):
                    cs = slice(128 * j, 128 * (j + 1))
                    nc.tensor.matmul(qp[:, :], t_wq[:, cs], tiles[j][:, :],
                                     start=(j == 0), stop=(j == NCT - 1))
                    if j % 2 == 1:
                        yield
                # evac: q20 rows 0:64 direct; q21 via staged shift; dup rows 64:127
                nc.scalar.activation(q20[b][0:64, ts], qp[0:64, :], COPY)
                qstg = sbS.tile([128, TB], F32R, name=f"qstg_{b}_{tb}", tag="qstg", bufs=1)
                nc.scalar.activation(qstg[64:128, :], qp[64:128, :], COPY)
                nc.sync.dma_start(out=q20[b][64:96, ts], in_=q20[b][0:32, ts])
                nc.sync.dma_start(out=q21[b][0:64, ts], in_=qstg[64:128, :])
                nc.sync.dma_start(out=q21[b][64:96, ts], in_=qstg[64:96, :])
                yield
                # ---- K ----
                kp = psX.tile([128, TB], F32, name=f"kp_{b}_{tb}", tag="x")
                for j in range(NCT):
                    cs = slice(128 * j, 128 * (j + 1))
                    nc.tensor.matmul(kp[:, :], t_wk[:, cs], tiles[j][:, :],
                                     start=(j == 0), stop=(j == NCT - 1))
                    if j % 2 == 1:
                        yield
                nc.vector.memset(kc0[b][96:97, ts].bitcast(F32), -1.0)
                nc.vector.memset(kc1[b][96:97, ts].bitcast(F32), -1.0)
                khi = sbS.tile([128, TB], F32R, name=f"khi_{b}_{tb}", tag="khi", bufs=1)
                klo = sbS.tile([128, TB], F32R, name=f"klo_{b}_{tb}", tag="klo", bufs=1)
                nc.vector.tensor_copy(khi[:, :], kp[:, :])
                nc.vector.tensor_sub(klo[:, :], kp[:, :], khi[:, :].bitcast(F32))
                nc.gpsimd.tensor_copy(kc0[b][0:64, ts], khi[0:64, :])
                nc.sync.dma_start(out=kc1[b][0:64, ts], in_=khi[64:128, :])
                nc.sync.dma_start(out=kc0[b][64:96, ts], in_=klo[0:32, :])
                nc.sync.dma_start(out=kc1[b][64:96, ts], in_=klo[64:96, :])
                yield
                # ---- V ----
                vp = psX.tile([128, TB], F32, name=f"vp_{b}_{tb}", tag="x")
                for j in range(NCT):
                    cs = slice(128 * j, 128 * (j + 1))
                    nc.tensor.matmul(vp[:, :], t_wv[:, cs], tiles[j][:, :],
                                     start=(j == 0), stop=(j == NCT - 1))
                    if j % 2 == 1:
                        yield
                vtr = sbS.tile([128, TB], F32R, name=f"vtr_{b}_{tb}", tag="vtr", bufs=1)
                nc.vector.tensor_copy(vtr[:, :], vp[:, :])
                yield
                for i in range(4):
                    st = 4 * tb + i
                    tvp = psX.tile([128, 128], F32R, name=f"tvp_{b}_{st}", tag="x")
                    nc.tensor.transpose(tvp[:, :], vtr[:, 128 * i:128 * (i + 1)], t_id[:, :])
                    va = vaug[b][st]
                    nc.gpsimd.memset(va[:, 64:65].bitcast(F32), 1.0)
                    nc.gpsimd.memset(va[:, 129:130].bitcast(F32), 1.0)
                    nc.scalar.activation(va[:, 0:64], tvp[:, 0:64], COPY)
                    nc.scalar.activation(va[:, 65:129], tvp[:, 64:128], COPY)
                    yield

            def gen_nat(b, tb):
                ts = slice(TB * tb, TB * (tb + 1))
                pend = []
                yield  # kc/q2 assembly DMAs land while the partner runs
                for i in range(4):
                    gt = 4 * tb + i
                    tts = slice(128 * gt, 128 * (gt + 1))
                    cols = 128 * (gt + 1)
                    nb = _ceildiv(cols, TB)
                    msc0 = sbS.tile([128, 128], F32R, name=f"msc0_{b}_{gt}", tag="msc0")
                    msc1 = sbS.tile([128, 128], F32R, name=f"msc1_{b}_{gt}", tag="msc1")
                    for j in range(nb):
                        w = min(TB, cols - TB * j)
                        ss = slice(TB * j, TB * j + w)
                        np0 = psX.tile([128, TB], F32, name=f"np0_{b}_{gt}_{j}", tag="x")
                        np1 = psX.tile([128, TB], F32, name=f"np1_{b}_{gt}_{j}", tag="x")
                        nc.tensor.matmul(np0[:, :w], q20[b][0:96, tts], kc0[b][0:96, ss],
                                         start=True, stop=True)
                        nc.tensor.matmul(np1[:, :w], q21[b][0:96, tts], kc1[b][0:96, ss],
                                         start=True, stop=True)
                        yield
                        if j == nb - 1:
                            dsl = slice(w - 128, w)
                            nc.vector.tensor_add(np0[:, dsl], np0[:, dsl], t_mnat[:, :])
                            nc.vector.tensor_add(np1[:, dsl], np1[:, dsl], t_mnat[:, :])
                        nc.vector.reduce_max(msc0[:, j:j + 1], np0[:, :w], axis=AX)
                        if GP_REDUCE:
                            nc.gpsimd.reduce_max(msc1[:, j:j + 1], np1[:, :w], axis=AX)
                        else:
                            nc.vector.reduce_max(msc1[:, j:j + 1], np1[:, :w], axis=AX)
                    nc.vector.reduce_max(msc0[:, 96:97], msc0[:, 0:nb], axis=AX)
                    if GP_REDUCE:
                        nc.gpsimd.reduce_max(msc1[:, 127:128], msc1[:, 0:nb], axis=AX)
                    else:
                        nc.vector.reduce_max(msc1[:, 96:97], msc1[:, 0:nb], axis=AX)
                    pend.append((i, msc0, msc1))
                    # delay the max transposes one t-tile so the reduces drain
                    if len(pend) >= 2:
                        yield from _emit_mtr(b, tb, pend.pop(0))
                while pend:
                    yield from _emit_mtr(b, tb, pend.pop(0))
                # m rows -> q2 row 127 (partition-aligned copies from psum)
                tps = mtrs.pop((b, tb))
                for k, (i, tp0, tp1) in enumerate(tps):
                    isl = slice(TB * tb + 128 * i, TB * tb + 128 * (i + 1))
                    nc.scalar.activation(q20[b][96:97, isl], tp0[96:97, :], COPY)
                    nc.scalar.activation(q21[b][96:97, isl], tp1[96:97, :], COPY)
                yield

            mtrs = {}

            def _emit_mtr(b, tb, item):
                i, msc0, msc1 = item
                tp0 = psX.tile([128, 128], F32R, name=f"tp0_{b}_{tb}_{i}", tag="x")
                tp1 = psX.tile([128, 128], F32R, name=f"tp1_{b}_{tb}_{i}", tag="x")
                nc.tensor.transpose(tp0[:, :], msc0[:, :], t_id[:, :])
                nc.tensor.transpose(tp1[:, :], msc1[:, :], t_id[:, :])
                mtrs.setdefault((b, tb), []).append((i, tp0, tp1))
                yield

            onorms = {}

            def gen_att(b, tb):
                if tb + 1 < NTB:
                    issue_x(b, tb + 1)
                for _ in range(3):
                    yield  # let the partner phase queue PE work ahead of sp0
                t0 = TB * tb
                ts = slice(t0, t0 + TB)
                ov0 = psO.tile([128, TB], F32, name=f"ov0_{b}_{tb}", tag="ov0")
                ov1 = psO.tile([128, TB], F32, name=f"ov1_{b}_{tb}", tag="ov1")
                last = 4 * (tb + 1) - 1
                pend_pv = None

                def _emit_pv(b_, tb_, st_, sl_, pt0_, pt1_, ov0_, ov1_):
                    nc.tensor.matmul(ov0_[0:65, sl_], vaug[b_][st_][:, 0:65], pt0_[:, sl_],
                                     start=(st_ == 0), stop=(st_ == last))
                    nc.tensor.matmul(ov1_[0:65, sl_], vaug[b_][st_][:, 65:130], pt1_[:, sl_],
                                     start=(st_ == 0), stop=(st_ == last))
                    yield

                for st in range(4 * (tb + 1)):
                    diag = st >= 4 * tb
                    coff = 128 * st - t0 if diag else 0
                    sl = slice(coff, TB)
                    tsl = slice(t0 + coff, t0 + TB)
                    sts = slice(128 * st, 128 * (st + 1))
                    sp0 = psS.tile([128, TB], F32, name=f"sp0_{b}_{tb}_{st}", tag="sp")
                    sp1 = psS.tile([128, TB], F32, name=f"sp1_{b}_{tb}_{st}", tag="sp")
                    pt0 = sbP.tile([128, TB], F32R, name=f"pt0_{b}_{tb}_{st}", tag="pt0")
                    pt1 = sbP.tile([128, TB], F32R, name=f"pt1_{b}_{tb}_{st}", tag="pt1")
                    nc.tensor.matmul(sp0[:, sl], kc0[b][0:97, sts], q20[b][0:97, tsl],
                                     start=True, stop=True)
                    if diag:
                        dsl = slice(coff, coff + 128)
                        nc.vector.tensor_add(sp0[:, dsl], sp0[:, dsl], t_mt[:, :])
                    nc.scalar.activation(pt0[:, sl], sp0[:, sl], EXP)
                    nc.tensor.matmul(sp1[:, sl], kc1[b][0:97, sts], q21[b][0:97, tsl],
                                     start=True, stop=True)
                    if diag:
                        dsl = slice(coff, coff + 128)
                        nc.vector.tensor_add(sp1[:, dsl], sp1[:, dsl], t_mt[:, :])
                    nc.scalar.activation(pt1[:, sl], sp1[:, sl], EXP)
                    yield
                    if pend_pv is not None:
                        yield from _emit_pv(*pend_pv)
                    pend_pv = (b, tb, st, sl, pt0, pt1, ov0, ov1)
                if pend_pv is not None:
                    yield from _emit_pv(*pend_pv)
                # ---- normalize ----
                # broadcast the denominators to [128,TB] via K=1 matmuls, then a
                # single 128-lane reciprocal (a [1,TB] reciprocal is ~6x slower)
                rcA = sbS.tile([65, TB], F32R, name=f"rcA_{b}_{tb}", tag="rcA", bufs=1)
                rcB = sbS.tile([65, TB], F32R, name=f"rcB_{b}_{tb}", tag="rcB", bufs=1)
                nc.vector.tensor_copy(rcA[64:65, :], ov0[64:65, :])
                nc.vector.tensor_copy(rcB[64:65, :], ov1[64:65, :])
                bc = psS.tile([128, TB], F32, name=f"bc_{b}_{tb}", tag="sp")
                nc.tensor.matmul(bc[:, :], ones65a[64:65, :], rcA[64:65, :],
                                 start=True, stop=False)
                nc.tensor.matmul(bc[:, :], ones65b[64:65, :], rcB[64:65, :],
                                 start=False, stop=True)
                rb_sb = sbS.tile([128, TB], F32, name=f"rb_{b}_{tb}", tag="rb", bufs=1)
                nc.vector.reciprocal_approx_fast(rb_sb[:, :], bc[:, :])
                ost = sbS.tile([128, TB], F32, name=f"ost_{b}_{tb}", tag="ost")
                stg1 = sbS.tile([64, TB], F32, name=f"stg1_{b}_{tb}", tag="stg1")
                nc.scalar.activation(ost[0:64, :], ov0[0:64, :], COPY)
                nc.scalar.activation(stg1[:, :], ov1[0:64, :], COPY)
                nc.sync.dma_start(out=ost[64:128, :], in_=stg1[:, :])
                onorm = sbS.tile([128, TB], F32R, name=f"onorm_{b}_{tb}", tag="onorm")
                nc.vector.tensor_mul(onorm[:, :], ost[:, :], rb_sb[:, :])
                yield
                onorms[(b, tb)] = onorm

            def gen_outproj(b, tb):
                onorm = onorms.pop((b, tb))
                # ---- output projection ----
                for tt in range(4):
                    gtt = 4 * tb + tt
                    tloc = slice(128 * tt, 128 * (tt + 1))
                    ysb = sbY.tile([128, C], BF16, name=f"ysb_{b}_{gtt}", tag="ysb")
                    for e in range(2):
                        es = slice(512 * e, 512 * (e + 1))
                        yp = psX.tile([128, 512], F32, name=f"yp_{b}_{gtt}_{e}", tag="x")
                        nc.tensor.matmul(yp[:, :], onorm[:, tloc], t_wp[:, es],
                                         start=True, stop=True)
                        if e == 0:
                            nc.scalar.activation(ysb[:, es], yp[:, :], COPY)
                        else:
                            nc.vector.tensor_copy(ysb[:, es], yp[:, :])
                        yield
                    eng = nc.gpsimd if tt % 2 == 0 else nc.sync
                    eng.dma_start(out=y[b, slice(128 * gtt, 128 * (gtt + 1)), :],
                                  in_=ysb[:, :])
                    yield

            # ---------------- phase-shifted schedule ----------------
            def make_gen(kind, b, tb):
                if kind == "P":
                    return gen_proj(b, tb)
                if kind == "N":
                    return gen_nat(b, tb)
                return gen_att(b, tb)

            stream0 = [(k, 0, tb) for tb in range(NTB) for k in ("P", "N", "S")]
            stream1 = [(k, 1, tb) for tb in range(NTB) for k in ("P", "N", "S")]
            WKIND = {"P": 2, "N": 1, "S": 3}
            nslots = len(stream0) + 2
            prev_s = []
            for i in range(nslots):
                gens, weights = [], []
                cur_s = []
                if i < len(stream0):
                    k, bb, tb = stream0[i]
                    gens.append(make_gen(k, bb, tb))
                    weights.append(WKIND[k])
                    if k == "S":
                        cur_s.append((bb, tb))
                if 1 <= i <= len(stream1):
                    k1, bb1, tb1 = stream1[i - 1]
                    gens.append(make_gen(k1, bb1, tb1))
                    weights.append(WKIND[k1])
                    if k1 == "S":
                        cur_s.append((bb1, tb1))
                for (bb, tb) in prev_s:
                    gens.append(gen_outproj(bb, tb))
                    weights.append(1)
                prev_s = cur_s
                _interleave(gens, weights)

    nc.compile()
    return nc


_NC_CACHE = {}


def _get_nc():
    if "nc" not in _NC_CACHE:
        _NC_CACHE["nc"] = build_nc()
    return _NC_CACHE["nc"]


def prepare_in_maps(ix, Wq, Wk, Wv, Wp):
    ix = np.asarray(ix, np.float32)
    Wq = np.asarray(Wq, np.float32)
    Wk = np.asarray(Wk, np.float32)
    Wv = np.asarray(Wv, np.float32)
    Wp = np.asarray(Wp, np.float32)

    xt = np.ascontiguousarray(ix.transpose(0, 2, 1))  # [B, C, T]
    mask = np.where(np.arange(128)[:, None] >= np.arange(128)[None, :], 0.0, NEG).astype(np.float32)

    in_maps = []
    for core in range(NCORES):
        h0, h1 = 2 * core, 2 * core + 1
        in_maps.append({
            "x": xt,
            "wq": _r12(np.concatenate([Wq[h0], Wq[h1]], axis=1) * 32.0),  # carries sqrt(C)
            "wk": _r12(np.concatenate([Wk[h0], Wk[h1]], axis=1)),
            "wv": _r12(np.concatenate([Wv[h0], Wv[h1]], axis=1)),
            "wp": _r12(Wp[D * h0:D * (h1 + 1), :]),
            "mask_nat": mask,
            "mask_t": np.ascontiguousarray(mask.T),
        })
    return in_maps


def kernel(ix, Wq, Wk, Wv, Wp, bp):
    in_maps = prepare_in_maps(ix, Wq, Wk, Wv, Wp)
    bp = np.asarray(bp, np.float32)
    nc = _get_nc()
    res = bass_utils.run_bass_kernel_spmd(nc, in_maps, list(range(NCORES)))
    out = np.zeros((B, T, C), np.float64)
    for r in res.results:
        out += np.asarray(r["y"], np.float32)
    out += bp
    return out.astype(np.float32)


# revision 32
# speedup vs baseline: 1.0628x; 1.0400x over previous
"""Trainium2 Bass kernel for nn_MultiHeadAttention_54580444397743.

Head-sharded tensor parallel over 8 NeuronCores: 2 heads/core, both batches.

Precision scheme (relies on fp32r PE operand rounding ~12 mantissa bits;
validated in numpy sim at ~4e-3 rel err vs 2e-2 gate):
  - projections: single fp32r MM per c-tile (W r12'd on host, x raw)
  - K split on-chip into hi/lo (12-bit chunks); kc = [Khi(64); Klo(63); -1]
  - scores^T in ONE K=128 MM: kc^T @ [Q; Q(63); m] = S - m (max folded in)
  - natural-orientation pass reuses the same operands for the row max
  - P^T = exp(S^T - m); PV via [V|1] augmented matmul gives out + denominator

Schedule: the two batches run phase-shifted (proj/nat/attn) with interleaved
instruction emission so the PE never idles (HAM stays at 2.4 GHz).
"""

import numpy as np

import concourse.bass as bass
import concourse.mybir as mybir
from concourse import bacc, bass_utils
from concourse.tile import TileContext
from concourse.masks import make_identity

B, T, C = 2, 2048, 1024
H, D = 16, 64
NCORES = 8
TB = 512                   # t-block width (PSUM bank)
NTB = T // TB              # 4 t-blocks per batch
NCT = C // 128             # 8 c-tiles
F32 = mybir.dt.float32
F32R = mybir.dt.float32r
NEG = -1.0e9
AX = mybir.AxisListType.X
COPY = mybir.ActivationFunctionType.Copy
EXP = mybir.ActivationFunctionType.Exp
BF16 = mybir.dt.bfloat16

GP_REDUCE = False   # Pool engine cannot do free-axis reduces
GP_YEVAC = False    # GPSIMD cannot access PSUM


def _r12(x):
    m, e = np.frexp(np.asarray(x, np.float64))
    return np.ldexp(np.round(m * 4096.0) / 4096.0, e).astype(np.float32)


def _ceildiv(a, b):
    return -(-a // b)


def _interleave(gens, weights=None):
    gens = [g for g in gens if g is not None]
    if weights is None:
        weights = [1] * len(gens)
    weights = list(weights[:len(gens)])
    while gens:
        alive, walive = [], []
        for g, w in zip(gens, weights):
            ok = True
            for _ in range(w):
                try:
                    next(g)
                except StopIteration:
                    ok = False
                    break
            if ok:
                alive.append(g)
                walive.append(w)
        gens, weights = alive, walive


def build_nc():
    nc = bacc.Bacc(target_bir_lowering=False, debug=False)

    x = nc.dram_tensor("x", [B, C, T], F32R, kind="ExternalInput")
    wq = nc.dram_tensor("wq", [C, 128], F32R, kind="ExternalInput")
    wk = nc.dram_tensor("wk", [C, 128], F32R, kind="ExternalInput")
    wv = nc.dram_tensor("wv", [C, 128], F32R, kind="ExternalInput")
    wp = nc.dram_tensor("wp", [128, C], F32R, kind="ExternalInput")
    mask_nat = nc.dram_tensor("mask_nat", [128, 128], F32, kind="ExternalInput")
    mask_t = nc.dram_tensor("mask_t", [128, 128], F32, kind="ExternalInput")
    y = nc.dram_tensor("y", [B, T, C], BF16, kind="ExternalOutput")

    with TileContext(nc) as tc:
        with (
            tc.tile_pool(name="sbW", bufs=1) as sbW,
            tc.tile_pool(name="sbBig", bufs=1) as sbBig,
            tc.tile_pool(name="sbX", bufs=16) as sbX,
            tc.tile_pool(name="sbS", bufs=2) as sbS,
            tc.tile_pool(name="sbP", bufs=4) as sbP,
            tc.tile_pool(name="sbY", bufs=2) as sbY,
            tc.tile_pool(name="psX", bufs=4, space="PSUM") as psX,
            tc.tile_pool(name="psS", bufs=2, space="PSUM") as psS,
            tc.tile_pool(name="psO", bufs=1, space="PSUM") as psO,
        ):
            xtiles = {}

            def issue_x(b, tb, eng=None):
                eng = eng or nc.sync
                ts = slice(TB * tb, TB * (tb + 1))
                tiles = []
                for j in range(NCT):
                    cs = slice(128 * j, 128 * (j + 1))
                    tx = sbX.tile([128, TB], F32R, name=f"x_{b}_{tb}_{j}", tag="x")
                    eng.dma_start(out=tx[:, :], in_=x[b, cs, ts])
                    tiles.append(tx)
                xtiles[(b, tb)] = tiles

            # ---------------- constants / weights ----------------
            t_wq = sbW.tile([128, C], F32R, name="t_wq")
            t_wk = sbW.tile([128, C], F32R, name="t_wk")
            t_wv = sbW.tile([128, C], F32R, name="t_wv")
            t_wp = sbW.tile([128, C], F32R, name="t_wp")
            issue_x(0, 0, eng=nc.gpsimd)
            for j in range(NCT):
                cs = slice(128 * j, 128 * (j + 1))
                nc.sync.dma_start(out=t_wq[:, cs], in_=wq[cs, :])
                nc.scalar.dma_start(out=t_wk[:, cs], in_=wk[cs, :])
                nc.gpsimd.dma_start(out=t_wv[:, cs], in_=wv[cs, :])
            nc.scalar.dma_start(out=t_wp[:, :], in_=wp[:, :])
            t_mnat = sbW.tile([128, 128], F32, name="t_mnat")
            t_mt = sbW.tile([128, 128], F32, name="t_mt")
            nc.scalar.dma_start(out=t_mnat[:, :], in_=mask_nat[:, :])
            nc.scalar.dma_start(out=t_mt[:, :], in_=mask_t[:, :])

            # warm the PE clock gate while the first x/w DMAs are in flight;
            # input is a memset tile so the chain is ready almost immediately
            t_wupin = sbW.tile([128, TB], F32R, name="t_wupin")
            nc.vector.memset(t_wupin[:, :].bitcast(F32), 0.5)
            for w in range(20):
                wup = psX.tile([128, TB], F32, name=f"wup_{w}", tag="x")
                nc.tensor.matmul(wup[:, :], t_wupin[:, 0:128], t_wupin[:, :],
                                 start=True, stop=True)
            t_idf = sbW.tile([128, 128], F32, name="t_idf")
            make_identity(nc, t_idf)
            t_id = sbW.tile([128, 128], F32R, name="t_id")
            nc.vector.tensor_copy(t_id[:, :], t_idf[:, :])
            # rows 64 of ones65a/b select head0/head1 cols for the recip bcast
            ones65a = sbW.tile([65, 128], F32R, name="ones65a")
            ones65b = sbW.tile([65, 128], F32R, name="ones65b")
            nc.vector.memset(ones65a[64:65, :].bitcast(F32), 0.0)
            nc.vector.memset(ones65b[64:65, :].bitcast(F32), 0.0)
            nc.vector.memset(ones65a[64:65, 0:64].bitcast(F32), 1.0)
            nc.vector.memset(ones65b[64:65, 64:128].bitcast(F32), 1.0)

            # ---------------- per-batch persistent tensors ----------------
            kc0, kc1, q20, q21, vaug = {}, {}, {}, {}, {}
            for b in range(B):
                kc0[b] = sbBig.tile([128, T], F32R, name=f"kc0_{b}", tag=f"kc0_{b}")
                kc1[b] = sbBig.tile([128, T], F32R, name=f"kc1_{b}", tag=f"kc1_{b}")
                q20[b] = sbBig.tile([128, T], F32R, name=f"q20_{b}", tag=f"q20_{b}")
                q21[b] = sbBig.tile([128, T], F32R, name=f"q21_{b}", tag=f"q21_{b}")
                vaug[b] = [
                    sbBig.tile([128, 130], F32R, name=f"vaug_{b}_{st}", tag=f"vaug_{b}_{st}")
                    for st in range(T // 128)
                ]

            # ---------------- phase generators ----------------
            def gen_proj(b, tb):
                if (b, tb) == (0, 0):
                    issue_x(1, 0)
                ts = slice(TB * tb, TB * (tb + 1))
                tiles = xtiles.pop((b, tb))
                # ---- Q ----
                qp = psX.tile([128, TB], F32, name=f"qp_{b}_{tb}", tag="x")
                for j in range(NCT):
                    cs = slice(128 * j, 128 * (j + 1))
                    nc.tensor.matmul(qp[:, :], t_wq[:, cs], tiles[j][:, :],
                                     start=(j == 0), stop=(j == NCT - 1))
                    if j % 2 == 1:
                        yield
                # evac: q20 rows 0:64 direct; q21 via staged shift; dup rows 64:127
                nc.scalar.activation(q20[b][0:64, ts], qp[0:64, :], COPY)
                qstg = sbS.tile([128, TB], F32R, name=f"qstg_{b}_{tb}", tag="qstg", bufs=1)
                nc.scalar.activation(qstg[64:128, :], qp[64:128, :], COPY)
                nc.sync.dma_start(out=q20[b][64:96, ts], in_=q20[b][0:32, ts])
                nc.sync.dma_start(out=q21[b][0:64, ts], in_=qstg[64:128, :])
                nc.sync.dma_start(out=q21[b][64:96, ts], in_=qstg[64:96, :])
                yield
                # ---- K ----
                kp = psX.tile([128, TB], F32, name=f"kp_{b}_{tb}", tag="x")
                for j in range(NCT):
                    cs = slice(128 * j, 128 * (j + 1))
                    nc.tensor.matmul(kp[:, :], t_wk[:, cs], tiles[j][:, :],
                                     start=(j == 0), stop=(j == NCT - 1))
                    if j % 2 == 1:
                        yield
                nc.vector.memset(kc0[b][96:97, ts].bitcast(F32), -1.0)
                nc.vector.memset(kc1[b][96:97, ts].bitcast(F32), -1.0)
                khi = sbS.tile([128, TB], F32R, name=f"khi_{b}_{tb}", tag="khi", bufs=1)
                klo = sbS.tile([128, TB], F32R, name=f"klo_{b}_{tb}", tag="klo", bufs=1)
                nc.vector.tensor_copy(khi[:, :], kp[:, :])
                nc.vector.tensor_sub(klo[:, :], kp[:, :], khi[:, :].bitcast(F32))
                nc.gpsimd.tensor_copy(kc0[b][0:64, ts], khi[0:64, :])
                nc.sync.dma_start(out=kc1[b][0:64, ts], in_=khi[64:128, :])
                nc.sync.dma_start(out=kc0[b][64:96, ts], in_=klo[0:32, :])
                nc.sync.dma_start(out=kc1[b][64:96, ts], in_=klo[64:96, :])
                yield
                # ---- V ----
                vp = psX.tile([128, TB], F32, name=f"vp_{b}_{tb}", tag="x")
                for j in range(NCT):
                    cs = slice(128 * j, 128 * (j + 1))
                    nc.tensor.matmul(vp[:, :], t_wv[:, cs], tiles[j][:, :],
                                     start=(j == 0), stop=(j == NCT - 1))
                    if j % 2 == 1:
                        yield
                vtr = sbS.tile([128, TB], F32R, name=f"vtr_{b}_{tb}", tag="vtr", bufs=1)
                nc.vector.tensor_copy(vtr[:, :], vp[:, :])
                yield
                for i in range(4):
                    st = 4 * tb + i
                    tvp = psX.tile([128, 128], F32R, name=f"tvp_{b}_{st}", tag="x")
                    nc.tensor.transpose(tvp[:, :], vtr[:, 128 * i:128 * (i + 1)], t_id[:, :])
                    va = vaug[b][st]
                    nc.gpsimd.memset(va[:, 64:65].bitcast(F32), 1.0)
                    nc.gpsimd.memset(va[:, 129:130].bitcast(F32), 1.0)
                    nc.scalar.activation(va[:, 0:64], tvp[:, 0:64], COPY)
                    nc.scalar.activation(va[:, 65:129], tvp[:, 64:128], COPY)
                    yield

            def gen_nat(b, tb):
                ts = slice(TB * tb, TB * (tb + 1))
                pend = []
                yield  # kc/q2 assembly DMAs land while the partner runs
                for i in range(4):
                    gt = 4 * tb + i
                    tts = slice(128 * gt, 128 * (gt + 1))
                    cols = 128 * (gt + 1)
                    nb = _ceildiv(cols, TB)
                    msc0 = sbS.tile([128, 128], F32R, name=f"msc0_{b}_{gt}", tag="msc0")
                    msc1 = sbS.tile([128, 128], F32R, name=f"msc1_{b}_{gt}", tag="msc1")
                    for j in range(nb):
                        w = min(TB, cols - TB * j)
                        ss = slice(TB * j, TB * j + w)
                        np0 = psX.tile([128, TB], F32, name=f"np0_{b}_{gt}_{j}", tag="x")
                        np1 = psX.tile([128, TB], F32, name=f"np1_{b}_{gt}_{j}", tag="x")
                        nc.tensor.matmul(np0[:, :w], q20[b][0:96, tts], kc0[b][0:96, ss],
                                         start=True, stop=True)
                        nc.tensor.matmul(np1[:, :w], q21[b][0:96, tts], kc1[b][0:96, ss],
                                         start=True, stop=True)
                        yield
                        if j == nb - 1:
                            dsl = slice(w - 128, w)
                            nc.vector.tensor_add(np0[:, dsl], np0[:, dsl], t_mnat[:, :])
                            nc.vector.tensor_add(np1[:, dsl], np1[:, dsl], t_mnat[:, :])
                        nc.vector.reduce_max(msc0[:, j:j + 1], np0[:, :w], axis=AX)
                        if GP_REDUCE:
                            nc.gpsimd.reduce_max(msc1[:, j:j + 1], np1[:, :w], axis=AX)
                        else:
                            nc.vector.reduce_max(msc1[:, j:j + 1], np1[:, :w], axis=AX)
                    nc.vector.reduce_max(msc0[:, 96:97], msc0[:, 0:nb], axis=AX)
                    if GP_REDUCE:
                        nc.gpsimd.reduce_max(msc1[:, 127:128], msc1[:, 0:nb], axis=AX)
                    else:
                        nc.vector.reduce_max(msc1[:, 96:97], msc1[:, 0:nb], axis=AX)
                    pend.append((i, msc0, msc1))
                    # delay the max transposes one t-tile so the reduces drain
                    if len(pend) >= 2:
                        yield from _emit_mtr(b, tb, pend.pop(0))
                while pend:
                    yield from _emit_mtr(b, tb, pend.pop(0))
                # m rows -> q2 row 127 (partition-aligned copies from psum)
                tps = mtrs.pop((b, tb))
                for k, (i, tp0, tp1) in enumerate(tps):
                    isl = slice(TB * tb + 128 * i, TB * tb + 128 * (i + 1))
                    nc.scalar.activation(q20[b][96:97, isl], tp0[96:97, :], COPY)
                    nc.scalar.activation(q21[b][96:97, isl], tp1[96:97, :], COPY)
                yield

            mtrs = {}

            def _emit_mtr(b, tb, item):
                i, msc0, msc1 = item
                tp0 = psX.tile([128, 128], F32R, name=f"tp0_{b}_{tb}_{i}", tag="x")
                tp1 = psX.tile([128, 128], F32R, name=f"tp1_{b}_{tb}_{i}", tag="x")
                nc.tensor.transpose(tp0[:, :], msc0[:, :], t_id[:, :])
                nc.tensor.transpose(tp1[:, :], msc1[:, :], t_id[:, :])
                mtrs.setdefault((b, tb), []).append((i, tp0, tp1))
                yield

            onorms = {}

            def gen_att(b, tb):
                if tb + 1 < NTB:
                    issue_x(b, tb + 1)
                for _ in range(3):
                    yield  # let the partner phase queue PE work ahead of sp0
                t0 = TB * tb
                ts = slice(t0, t0 + TB)
                ov0 = psO.tile([128, TB], F32, name=f"ov0_{b}_{tb}", tag="ov0")
                ov1 = psO.tile([128, TB], F32, name=f"ov1_{b}_{tb}", tag="ov1")
                last = 4 * (tb + 1) - 1
                pend_pv = None

                def _emit_pv(b_, tb_, st_, sl_, pt0_, pt1_, ov0_, ov1_):
                    nc.tensor.matmul(ov0_[0:65, sl_], vaug[b_][st_][:, 0:65], pt0_[:, sl_],
                                     start=(st_ == 0), stop=(st_ == last))
                    nc.tensor.matmul(ov1_[0:65, sl_], vaug[b_][st_][:, 65:130], pt1_[:, sl_],
                                     start=(st_ == 0), stop=(st_ == last))
                    yield

                for st in range(4 * (tb + 1)):
                    diag = st >= 4 * tb
                    coff = 128 * st - t0 if diag else 0
                    sl = slice(coff, TB)
                    tsl = slice(t0 + coff, t0 + TB)
                    sts = slice(128 * st, 128 * (st + 1))
                    sp0 = psS.tile([128, TB], F32, name=f"sp0_{b}_{tb}_{st}", tag="sp")
                    sp1 = psS.tile([128, TB], F32, name=f"sp1_{b}_{tb}_{st}", tag="sp")
                    pt0 = sbP.tile([128, TB], F32R, name=f"pt0_{b}_{tb}_{st}", tag="pt0")
                    pt1 = sbP.tile([128, TB], F32R, name=f"pt1_{b}_{tb}_{st}", tag="pt1")
                    nc.tensor.matmul(sp0[:, sl], kc0[b][0:97, sts], q20[b][0:97, tsl],
                                     start=True, stop=True)
                    if diag:
                        dsl = slice(coff, coff + 128)
                        nc.vector.tensor_add(sp0[:, dsl], sp0[:, dsl], t_mt[:, :])
                    nc.scalar.activation(pt0[:, sl], sp0[:, sl], EXP)
                    nc.tensor.matmul(sp1[:, sl], kc1[b][0:97, sts], q21[b][0:97, tsl],
                                     start=True, stop=True)
                    if diag:
                        dsl = slice(coff, coff + 128)
                        nc.vector.tensor_add(sp1[:, dsl], sp1[:, dsl], t_mt[:, :])
                    nc.scalar.activation(pt1[:, sl], sp1[:, sl], EXP)
                    yield
                    if pend_pv is not None:
                        yield from _emit_pv(*pend_pv)
                    pend_pv = (b, tb, st, sl, pt0, pt1, ov0, ov1)
                if pend_pv is not None:
                    yield from _emit_pv(*pend_pv)
                # ---- normalize ----
                # broadcast the denominators to [128,TB] via K=1 matmuls, then a
                # single 128-lane reciprocal (a [1,TB] reciprocal is ~6x slower)
                rcA = sbS.tile([65, TB], F32R, name=f"rcA_{b}_{tb}", tag="rcA", bufs=1)
                rcB = sbS.tile([65, TB], F32R, name=f"rcB_{b}_{tb}", tag="rcB", bufs=1)
                nc.vector.tensor_copy(rcA[64:65, :], ov0[64:65, :])
                nc.vector.tensor_copy(rcB[64:65, :], ov1[64:65, :])
                bc = psS.tile([128, TB], F32, name=f"bc_{b}_{tb}", tag="sp")
                nc.tensor.matmul(bc[:, :], ones65a[64:65, :], rcA[64:65, :],
                                 start=True, stop=False)
                nc.tensor.matmul(bc[:, :], ones65b[64:65, :], rcB[64:65, :],
                                 start=False, stop=True)
                rb_sb = sbS.tile([128, TB], F32, name=f"rb_{b}_{tb}", tag="rb", bufs=1)
                nc.vector.reciprocal_approx_fast(rb_sb[:, :], bc[:, :])
                ost = sbS.tile([128, TB], F32, name=f"ost_{b}_{tb}", tag="ost")
                stg1 = sbS.tile([64, TB], F32, name=f"stg1_{b}_{tb}", tag="stg1")
                nc.scalar.activation(ost[0:64, :], ov0[0:64, :], COPY)
                nc.scalar.activation(stg1[:, :], ov1[0:64, :], COPY)
                nc.sync.dma_start(out=ost[64:128, :], in_=stg1[:, :])
                onorm = sbS.tile([128, TB], F32R, name=f"onorm_{b}_{tb}", tag="onorm")
                nc.vector.tensor_mul(onorm[:, :], ost[:, :], rb_sb[:, :])
                yield
                onorms[(b, tb)] = onorm

            def gen_outproj(b, tb):
                onorm = onorms.pop((b, tb))
                # ---- output projection ----
                for tt in range(4):
                    gtt = 4 * tb + tt
                    tloc = slice(128 * tt, 128 * (tt + 1))
                    ysb = sbY.tile([128, C], BF16, name=f"ysb_{b}_{gtt}", tag="ysb")
                    for e in range(2):
                        es = slice(512 * e, 512 * (e + 1))
                        yp = psX.tile([128, 512], F32, name=f"yp_{b}_{gtt}_{e}", tag="x")
                        nc.tensor.matmul(yp[:, :], onorm[:, tloc], t_wp[:, es],
                                         start=True, stop=True)
                        if e == 0:
                            nc.scalar.activation(ysb[:, es], yp[:, :], COPY)
                        else:
                            nc.vector.tensor_copy(ysb[:, es], yp[:, :])
                        yield
                    eng = nc.gpsimd if tt % 2 == 0 else nc.sync
                    eng.dma_start(out=y[b, slice(128 * gtt, 128 * (gtt + 1)), :],
                                  in_=ysb[:, :])
                    yield

            # ---------------- phase-shifted schedule ----------------
            def make_gen(kind, b, tb):
                if kind == "P":
                    return gen_proj(b, tb)
                if kind == "N":
                    return gen_nat(b, tb)
                return gen_att(b, tb)

            stream0 = [(k, 0, tb) for tb in range(NTB) for k in ("P", "N", "S")]
            stream1 = [(k, 1, tb) for tb in range(NTB) for k in ("P", "N", "S")]
            WKIND = {"P": 2, "N": 1, "S": 3}
            nslots = len(stream0) + 2
            prev_s = []
            for i in range(nslots):
                gens, weights = [], []
                cur_s = []
                if i < len(stream0):
                    k, bb, tb = stream0[i]
                    gens.append(make_gen(k, bb, tb))
                    weights.append(WKIND[k])
                    if k == "S":
                        cur_s.append((bb, tb))
                if 1 <= i <= len(stream1):
                    k1, bb1, tb1 = stream1[i - 1]
                    gens.append(make_gen(k1, bb1, tb1))
                    weights.append(WKIND[k1])
                    if k1 == "S":
                        cur_s.append((bb1, tb1))
                for (bb, tb) in prev_s:
                    gens.append(gen_outproj(bb, tb))
                    weights.append(1)
                prev_s = cur_s
                _interleave(gens, weights)

    nc.compile()
    return nc


_NC_CACHE = {}


def _get_nc():
    if "nc" not in _NC_CACHE:
        _NC_CACHE["nc"] = build_nc()
    return _NC_CACHE["nc"]


def prepare_in_maps(ix, Wq, Wk, Wv, Wp):
    ix = np.asarray(ix, np.float32)
    Wq = np.asarray(Wq, np.float32)
    Wk = np.asarray(Wk, np.float32)
    Wv = np.asarray(Wv, np.float32)
    Wp = np.asarray(Wp, np.float32)

    xt = np.ascontiguousarray(ix.transpose(0, 2, 1))  # [B, C, T]
    mask = np.where(np.arange(128)[:, None] >= np.arange(128)[None, :], 0.0, NEG).astype(np.float32)

    in_maps = []
    for core in range(NCORES):
        h0, h1 = 2 * core, 2 * core + 1
        in_maps.append({
            "x": xt,
            "wq": _r12(np.concatenate([Wq[h0], Wq[h1]], axis=1) * 32.0),  # carries sqrt(C)
            "wk": _r12(np.concatenate([Wk[h0], Wk[h1]], axis=1)),
            "wv": _r12(np.concatenate([Wv[h0], Wv[h1]], axis=1)),
            "wp": _r12(Wp[D * h0:D * (h1 + 1), :]),
            "mask_nat": mask,
            "mask_t": np.ascontiguousarray(mask.T),
        })
    return in_maps


def kernel(ix, Wq, Wk, Wv, Wp, bp):
    in_maps = prepare_in_maps(ix, Wq, Wk, Wv, Wp)
    bp = np.asarray(bp, np.float32)
    nc = _get_nc()
    res = bass_utils.run_bass_kernel_spmd(nc, in_maps, list(range(NCORES)))
    out = np.zeros((B, T, C), np.float64)
    for r in res.results:
        out += np.asarray(r["y"], np.float32)
    out += bp
    return out.astype(np.float32)


# revision 34
# speedup vs baseline: 1.0655x; 1.0026x over previous
"""Trainium2 Bass kernel for nn_MultiHeadAttention_54580444397743.

Head-sharded tensor parallel over 8 NeuronCores: 2 heads/core, both batches.

Precision scheme (relies on fp32r PE operand rounding ~12 mantissa bits;
validated in numpy sim at ~4e-3 rel err vs 2e-2 gate):
  - projections: single fp32r MM per c-tile (W r12'd on host, x raw)
  - K split on-chip into hi/lo (12-bit chunks); kc = [Khi(64); Klo(63); -1]
  - scores^T in ONE K=128 MM: kc^T @ [Q; Q(63); m] = S - m (max folded in)
  - natural-orientation pass reuses the same operands for the row max
  - P^T = exp(S^T - m); PV via [V|1] augmented matmul gives out + denominator

Schedule: the two batches run phase-shifted (proj/nat/attn) with interleaved
instruction emission so the PE never idles (HAM stays at 2.4 GHz).
"""

import numpy as np

import concourse.bass as bass
import concourse.mybir as mybir
from concourse import bacc, bass_utils
from concourse.tile import TileContext
from concourse.masks import make_identity

B, T, C = 2, 2048, 1024
H, D = 16, 64
NCORES = 8
TB = 512                   # t-block width (PSUM bank)
NTB = T // TB              # 4 t-blocks per batch
NCT = C // 128             # 8 c-tiles
F32 = mybir.dt.float32
F32R = mybir.dt.float32r
NEG = -1.0e9
AX = mybir.AxisListType.X
COPY = mybir.ActivationFunctionType.Copy
EXP = mybir.ActivationFunctionType.Exp
BF16 = mybir.dt.bfloat16

GP_REDUCE = False   # Pool engine cannot do free-axis reduces
GP_YEVAC = False    # GPSIMD cannot access PSUM


def _r12(x):
    m, e = np.frexp(np.asarray(x, np.float64))
    return np.ldexp(np.round(m * 4096.0) / 4096.0, e).astype(np.float32)


def _ceildiv(a, b):
    return -(-a // b)


def _interleave(gens, weights=None):
    gens = [g for g in gens if g is not None]
    if weights is None:
        weights = [1] * len(gens)
    weights = list(weights[:len(gens)])
    while gens:
        alive, walive = [], []
        for g, w in zip(gens, weights):
            ok = True
            for _ in range(w):
                try:
                    next(g)
                except StopIteration:
                    ok = False
                    break
            if ok:
                alive.append(g)
                walive.append(w)
        gens, weights = alive, walive


def build_nc():
    nc = bacc.Bacc(target_bir_lowering=False, debug=False)

    x = nc.dram_tensor("x", [B, C, T], F32R, kind="ExternalInput")
    wq = nc.dram_tensor("wq", [C, 128], F32R, kind="ExternalInput")
    wk = nc.dram_tensor("wk", [C, 128], F32R, kind="ExternalInput")
    wv = nc.dram_tensor("wv", [C, 128], F32R, kind="ExternalInput")
    wp = nc.dram_tensor("wp", [128, C], F32R, kind="ExternalInput")
    mask_nat = nc.dram_tensor("mask_nat", [128, 128], F32, kind="ExternalInput")
    mask_t = nc.dram_tensor("mask_t", [128, 128], F32, kind="ExternalInput")
    y = nc.dram_tensor("y", [B, T, C], BF16, kind="ExternalOutput")

    with TileContext(nc) as tc:
        with (
            tc.tile_pool(name="sbW", bufs=1) as sbW,
            tc.tile_pool(name="sbBig", bufs=1) as sbBig,
            tc.tile_pool(name="sbX", bufs=16) as sbX,
            tc.tile_pool(name="sbS", bufs=2) as sbS,
            tc.tile_pool(name="sbP", bufs=4) as sbP,
            tc.tile_pool(name="sbY", bufs=2) as sbY,
            tc.tile_pool(name="psX", bufs=4, space="PSUM") as psX,
            tc.tile_pool(name="psS", bufs=2, space="PSUM") as psS,
            tc.tile_pool(name="psO", bufs=1, space="PSUM") as psO,
        ):
            xtiles = {}

            def issue_x(b, tb, eng=None):
                eng = eng or nc.sync
                ts = slice(TB * tb, TB * (tb + 1))
                tiles = []
                for j in range(NCT):
                    cs = slice(128 * j, 128 * (j + 1))
                    tx = sbX.tile([128, TB], F32R, name=f"x_{b}_{tb}_{j}", tag="x")
                    eng.dma_start(out=tx[:, :], in_=x[b, cs, ts])
                    tiles.append(tx)
                xtiles[(b, tb)] = tiles

            # ---------------- constants / weights ----------------
            t_wq = sbW.tile([128, C], F32R, name="t_wq")
            t_wk = sbW.tile([128, C], F32R, name="t_wk")
            t_wv = sbW.tile([128, C], F32R, name="t_wv")
            t_wp = sbW.tile([128, C], F32R, name="t_wp")
            issue_x(0, 0, eng=nc.gpsimd)
            for j in range(NCT):
                cs = slice(128 * j, 128 * (j + 1))
                nc.sync.dma_start(out=t_wq[:, cs], in_=wq[cs, :])
                nc.scalar.dma_start(out=t_wk[:, cs], in_=wk[cs, :])
                nc.gpsimd.dma_start(out=t_wv[:, cs], in_=wv[cs, :])
            nc.scalar.dma_start(out=t_wp[:, :], in_=wp[:, :])
            t_mnat = sbW.tile([128, 128], F32, name="t_mnat")
            t_mt = sbW.tile([128, 128], F32, name="t_mt")
            nc.scalar.dma_start(out=t_mnat[:, :], in_=mask_nat[:, :])
            nc.scalar.dma_start(out=t_mt[:, :], in_=mask_t[:, :])

            # warm the PE clock gate while the first x/w DMAs are in flight;
            # input is a memset tile so the chain is ready almost immediately
            t_wupin = sbW.tile([128, TB], F32R, name="t_wupin")
            nc.vector.memset(t_wupin[:, :].bitcast(F32), 0.5)
            for w in range(20):
                wup = psX.tile([128, TB], F32, name=f"wup_{w}", tag="x")
                nc.tensor.matmul(wup[:, :], t_wupin[:, 0:128], t_wupin[:, :],
                                 start=True, stop=True)
            t_idf = sbW.tile([128, 128], F32, name="t_idf")
            make_identity(nc, t_idf)
            t_id = sbW.tile([128, 128], F32R, name="t_id")
            nc.vector.tensor_copy(t_id[:, :], t_idf[:, :])
            # rows 64 of ones65a/b select head0/head1 cols for the recip bcast
            ones65a = sbW.tile([65, 128], F32R, name="ones65a")
            ones65b = sbW.tile([65, 128], F32R, name="ones65b")
            nc.vector.memset(ones65a[64:65, :].bitcast(F32), 0.0)
            nc.vector.memset(ones65b[64:65, :].bitcast(F32), 0.0)
            nc.vector.memset(ones65a[64:65, 0:64].bitcast(F32), 1.0)
            nc.vector.memset(ones65b[64:65, 64:128].bitcast(F32), 1.0)

            # ---------------- per-batch persistent tensors ----------------
            kc0, kc1, q20, q21, vaug = {}, {}, {}, {}, {}
            for b in range(B):
                kc0[b] = sbBig.tile([128, T], F32R, name=f"kc0_{b}", tag=f"kc0_{b}")
                kc1[b] = sbBig.tile([128, T], F32R, name=f"kc1_{b}", tag=f"kc1_{b}")
                q20[b] = sbBig.tile([128, T], F32R, name=f"q20_{b}", tag=f"q20_{b}")
                q21[b] = sbBig.tile([128, T], F32R, name=f"q21_{b}", tag=f"q21_{b}")
                vaug[b] = [
                    sbBig.tile([128, 130], F32R, name=f"vaug_{b}_{st}", tag=f"vaug_{b}_{st}")
                    for st in range(T // 128)
                ]

            # ---------------- phase generators ----------------
            def gen_proj(b, tb):
                if (b, tb) == (0, 0):
                    issue_x(1, 0)
                if tb + 1 < NTB:
                    issue_x(b, tb + 1)
                ts = slice(TB * tb, TB * (tb + 1))
                tiles = xtiles.pop((b, tb))
                # ---- Q ----
                qp = psX.tile([128, TB], F32, name=f"qp_{b}_{tb}", tag="x")
                for j in range(NCT):
                    cs = slice(128 * j, 128 * (j + 1))
                    nc.tensor.matmul(qp[:, :], t_wq[:, cs], tiles[j][:, :],
                                     start=(j == 0), stop=(j == NCT - 1))
                    if j % 2 == 1:
                        yield
                # evac: q20 rows 0:64 direct; q21 via staged shift; dup rows 64:127
                nc.scalar.activation(q20[b][0:64, ts], qp[0:64, :], COPY)
                qstg = sbS.tile([128, TB], F32R, name=f"qstg_{b}_{tb}", tag="qstg", bufs=1)
                nc.scalar.activation(qstg[64:128, :], qp[64:128, :], COPY)
                nc.sync.dma_start(out=q20[b][64:96, ts], in_=q20[b][0:32, ts])
                nc.sync.dma_start(out=q21[b][0:64, ts], in_=qstg[64:128, :])
                nc.sync.dma_start(out=q21[b][64:96, ts], in_=qstg[64:96, :])
                yield
                # ---- K ----
                kp = psX.tile([128, TB], F32, name=f"kp_{b}_{tb}", tag="x")
                for j in range(NCT):
                    cs = slice(128 * j, 128 * (j + 1))
                    nc.tensor.matmul(kp[:, :], t_wk[:, cs], tiles[j][:, :],
                                     start=(j == 0), stop=(j == NCT - 1))
                    if j % 2 == 1:
                        yield
                nc.vector.memset(kc0[b][96:97, ts].bitcast(F32), -1.0)
                nc.vector.memset(kc1[b][96:97, ts].bitcast(F32), -1.0)
                khi = sbS.tile([128, TB], F32R, name=f"khi_{b}_{tb}", tag="khi", bufs=1)
                klo = sbS.tile([128, TB], F32R, name=f"klo_{b}_{tb}", tag="klo", bufs=1)
                nc.vector.tensor_copy(khi[:, :], kp[:, :])
                nc.vector.tensor_sub(klo[:, :], kp[:, :], khi[:, :].bitcast(F32))
                nc.gpsimd.tensor_copy(kc0[b][0:64, ts], khi[0:64, :])
                nc.sync.dma_start(out=kc1[b][0:64, ts], in_=khi[64:128, :])
                nc.sync.dma_start(out=kc0[b][64:96, ts], in_=klo[0:32, :])
                nc.sync.dma_start(out=kc1[b][64:96, ts], in_=klo[64:96, :])
                yield
                # ---- V ----
                vp = psX.tile([128, TB], F32, name=f"vp_{b}_{tb}", tag="x")
                for j in range(NCT):
                    cs = slice(128 * j, 128 * (j + 1))
                    nc.tensor.matmul(vp[:, :], t_wv[:, cs], tiles[j][:, :],
                                     start=(j == 0), stop=(j == NCT - 1))
                    if j % 2 == 1:
                        yield
                vtr = sbS.tile([128, TB], F32R, name=f"vtr_{b}_{tb}", tag="vtr", bufs=1)
                nc.vector.tensor_copy(vtr[:, :], vp[:, :])
                yield
                for i in range(4):
                    st = 4 * tb + i
                    tvp = psX.tile([128, 128], F32R, name=f"tvp_{b}_{st}", tag="x")
                    nc.tensor.transpose(tvp[:, :], vtr[:, 128 * i:128 * (i + 1)], t_id[:, :])
                    va = vaug[b][st]
                    nc.gpsimd.memset(va[:, 64:65].bitcast(F32), 1.0)
                    nc.gpsimd.memset(va[:, 129:130].bitcast(F32), 1.0)
                    nc.scalar.activation(va[:, 0:64], tvp[:, 0:64], COPY)
                    nc.scalar.activation(va[:, 65:129], tvp[:, 64:128], COPY)
                    yield

            def gen_nat(b, tb):
                ts = slice(TB * tb, TB * (tb + 1))
                pend = []
                yield  # kc/q2 assembly DMAs land while the partner runs
                for i in range(4):
                    gt = 4 * tb + i
                    tts = slice(128 * gt, 128 * (gt + 1))
                    cols = 128 * (gt + 1)
                    nb = _ceildiv(cols, TB)
                    msc0 = sbS.tile([128, 128], F32R, name=f"msc0_{b}_{gt}", tag="msc0")
                    msc1 = sbS.tile([128, 128], F32R, name=f"msc1_{b}_{gt}", tag="msc1")
                    for j in range(nb):
                        w = min(TB, cols - TB * j)
                        ss = slice(TB * j, TB * j + w)
                        np0 = psX.tile([128, TB], F32, name=f"np0_{b}_{gt}_{j}", tag="x")
                        np1 = psX.tile([128, TB], F32, name=f"np1_{b}_{gt}_{j}", tag="x")
                        nc.tensor.matmul(np0[:, :w], q20[b][0:96, tts], kc0[b][0:96, ss],
                                         start=True, stop=True)
                        nc.tensor.matmul(np1[:, :w], q21[b][0:96, tts], kc1[b][0:96, ss],
                                         start=True, stop=True)
                        yield
                        if j == nb - 1:
                            dsl = slice(w - 128, w)
                            nc.vector.tensor_add(np0[:, dsl], np0[:, dsl], t_mnat[:, :])
                            nc.vector.tensor_add(np1[:, dsl], np1[:, dsl], t_mnat[:, :])
                        nc.vector.reduce_max(msc0[:, j:j + 1], np0[:, :w], axis=AX)
                        if GP_REDUCE:
                            nc.gpsimd.reduce_max(msc1[:, j:j + 1], np1[:, :w], axis=AX)
                        else:
                            nc.vector.reduce_max(msc1[:, j:j + 1], np1[:, :w], axis=AX)
                    nc.vector.reduce_max(msc0[:, 96:97], msc0[:, 0:nb], axis=AX)
                    if GP_REDUCE:
                        nc.gpsimd.reduce_max(msc1[:, 127:128], msc1[:, 0:nb], axis=AX)
                    else:
                        nc.vector.reduce_max(msc1[:, 96:97], msc1[:, 0:nb], axis=AX)
                    pend.append((i, msc0, msc1))
                    # delay the max transposes one t-tile so the reduces drain
                    if len(pend) >= 2:
                        yield from _emit_mtr(b, tb, pend.pop(0))
                while pend:
                    yield from _emit_mtr(b, tb, pend.pop(0))
                # m rows -> q2 row 127 (partition-aligned copies from psum)
                tps = mtrs.pop((b, tb))
                for k, (i, tp0, tp1) in enumerate(tps):
                    isl = slice(TB * tb + 128 * i, TB * tb + 128 * (i + 1))
                    nc.scalar.activation(q20[b][96:97, isl], tp0[96:97, :], COPY)
                    nc.scalar.activation(q21[b][96:97, isl], tp1[96:97, :], COPY)
                yield

            mtrs = {}

            def _emit_mtr(b, tb, item):
                i, msc0, msc1 = item
                tp0 = psX.tile([128, 128], F32R, name=f"tp0_{b}_{tb}_{i}", tag="x")
                tp1 = psX.tile([128, 128], F32R, name=f"tp1_{b}_{tb}_{i}", tag="x")
                nc.tensor.transpose(tp0[:, :], msc0[:, :], t_id[:, :])
                nc.tensor.transpose(tp1[:, :], msc1[:, :], t_id[:, :])
                mtrs.setdefault((b, tb), []).append((i, tp0, tp1))
                yield

            onorms = {}

            def gen_att(b, tb):
                for _ in range(3):
                    yield  # let the partner phase queue PE work ahead of sp0
                t0 = TB * tb
                ts = slice(t0, t0 + TB)
                ov0 = psO.tile([128, TB], F32, name=f"ov0_{b}_{tb}", tag="ov0")
                ov1 = psO.tile([128, TB], F32, name=f"ov1_{b}_{tb}", tag="ov1")
                last = 4 * (tb + 1) - 1
                pend_pv = None

                def _emit_pv(b_, tb_, st_, sl_, pt0_, pt1_, ov0_, ov1_):
                    nc.tensor.matmul(ov0_[0:65, sl_], vaug[b_][st_][:, 0:65], pt0_[:, sl_],
                                     start=(st_ == 0), stop=(st_ == last))
                    nc.tensor.matmul(ov1_[0:65, sl_], vaug[b_][st_][:, 65:130], pt1_[:, sl_],
                                     start=(st_ == 0), stop=(st_ == last))
                    yield

                for st in range(4 * (tb + 1)):
                    diag = st >= 4 * tb
                    coff = 128 * st - t0 if diag else 0
                    sl = slice(coff, TB)
                    tsl = slice(t0 + coff, t0 + TB)
                    sts = slice(128 * st, 128 * (st + 1))
                    sp0 = psS.tile([128, TB], F32, name=f"sp0_{b}_{tb}_{st}", tag="sp")
                    sp1 = psS.tile([128, TB], F32, name=f"sp1_{b}_{tb}_{st}", tag="sp")
                    pt0 = sbP.tile([128, TB], F32R, name=f"pt0_{b}_{tb}_{st}", tag="pt0")
                    pt1 = sbP.tile([128, TB], F32R, name=f"pt1_{b}_{tb}_{st}", tag="pt1")
                    nc.tensor.matmul(sp0[:, sl], kc0[b][0:97, sts], q20[b][0:97, tsl],
                                     start=True, stop=True)
                    if diag:
                        dsl = slice(coff, coff + 128)
                        nc.vector.tensor_add(sp0[:, dsl], sp0[:, dsl], t_mt[:, :])
                    nc.scalar.activation(pt0[:, sl], sp0[:, sl], EXP)
                    nc.tensor.matmul(sp1[:, sl], kc1[b][0:97, sts], q21[b][0:97, tsl],
                                     start=True, stop=True)
                    if diag:
                        dsl = slice(coff, coff + 128)
                        nc.vector.tensor_add(sp1[:, dsl], sp1[:, dsl], t_mt[:, :])
                    nc.scalar.activation(pt1[:, sl], sp1[:, sl], EXP)
                    yield
                    if pend_pv is not None:
                        yield from _emit_pv(*pend_pv)
                    pend_pv = (b, tb, st, sl, pt0, pt1, ov0, ov1)
                if pend_pv is not None:
                    yield from _emit_pv(*pend_pv)
                # ---- normalize ----
                # broadcast the denominators to [128,TB] via K=1 matmuls, then a
                # single 128-lane reciprocal (a [1,TB] reciprocal is ~6x slower)
                rcA = sbS.tile([65, TB], F32R, name=f"rcA_{b}_{tb}", tag="rcA", bufs=1)
                rcB = sbS.tile([65, TB], F32R, name=f"rcB_{b}_{tb}", tag="rcB", bufs=1)
                nc.vector.tensor_copy(rcA[64:65, :], ov0[64:65, :])
                nc.vector.tensor_copy(rcB[64:65, :], ov1[64:65, :])
                bc = psS.tile([128, TB], F32, name=f"bc_{b}_{tb}", tag="sp")
                nc.tensor.matmul(bc[:, :], ones65a[64:65, :], rcA[64:65, :],
                                 start=True, stop=False)
                nc.tensor.matmul(bc[:, :], ones65b[64:65, :], rcB[64:65, :],
                                 start=False, stop=True)
                rb_sb = sbS.tile([128, TB], F32, name=f"rb_{b}_{tb}", tag="rb", bufs=1)
                nc.vector.reciprocal_approx_fast(rb_sb[:, :], bc[:, :])
                ost = sbS.tile([128, TB], F32, name=f"ost_{b}_{tb}", tag="ost")
                stg1 = sbS.tile([64, TB], F32, name=f"stg1_{b}_{tb}", tag="stg1")
                nc.scalar.activation(ost[0:64, :], ov0[0:64, :], COPY)
                nc.scalar.activation(stg1[:, :], ov1[0:64, :], COPY)
                nc.sync.dma_start(out=ost[64:128, :], in_=stg1[:, :])
                onorm = sbS.tile([128, TB], F32R, name=f"onorm_{b}_{tb}", tag="onorm")
                for q in range(4):
                    qs = slice(128 * q, 128 * (q + 1))
                    nc.vector.tensor_mul(onorm[:, qs], ost[:, qs], rb_sb[:, qs])
                yield
                onorms[(b, tb)] = onorm

            def gen_outproj(b, tb):
                onorm = onorms.pop((b, tb))
                # ---- output projection ----
                for tt in range(4):
                    gtt = 4 * tb + tt
                    tloc = slice(128 * tt, 128 * (tt + 1))
                    ysb = sbY.tile([128, C], BF16, name=f"ysb_{b}_{gtt}", tag="ysb")
                    for e in range(2):
                        es = slice(512 * e, 512 * (e + 1))
                        yp = psX.tile([128, 512], F32, name=f"yp_{b}_{gtt}_{e}", tag="x")
                        nc.tensor.matmul(yp[:, :], onorm[:, tloc], t_wp[:, es],
                                         start=True, stop=True)
                        if e == 0:
                            nc.scalar.activation(ysb[:, es], yp[:, :], COPY)
                        else:
                            nc.vector.tensor_copy(ysb[:, es], yp[:, :])
                        yield
                    eng = nc.gpsimd if tt % 2 == 0 else nc.sync
                    eng.dma_start(out=y[b, slice(128 * gtt, 128 * (gtt + 1)), :],
                                  in_=ysb[:, :])
                    yield

            # ---------------- phase-shifted schedule ----------------
            def make_gen(kind, b, tb):
                if kind == "P":
                    return gen_proj(b, tb)
                if kind == "N":
                    return gen_nat(b, tb)
                return gen_att(b, tb)

            stream0 = [(k, 0, tb) for tb in range(NTB) for k in ("P", "N", "S")]
            stream1 = [(k, 1, tb) for tb in range(NTB) for k in ("P", "N", "S")]
            # slot k: P[k] | N[k-2] | S[k-4] | outproj of S[k-5] — a 3-stage
            # software pipeline so every slot has dense proj MMs to fill the
            # DVE-bound nat and ACT-bound attention stalls
            plist = [(bb, tb) for tb in range(NTB) for bb in range(B)]
            nslots = len(plist) + 5
            for i in range(nslots):
                gens, weights = [], []
                if i < len(plist):
                    gens.append(gen_proj(*plist[i]))
                    weights.append(2)
                if 0 <= i - 2 < len(plist):
                    gens.append(gen_nat(*plist[i - 2]))
                    weights.append(1)
                if 0 <= i - 4 < len(plist):
                    gens.append(gen_att(*plist[i - 4]))
                    weights.append(3)
                if 0 <= i - 5 < len(plist):
                    gens.append(gen_outproj(*plist[i - 5]))
                    weights.append(1)
                _interleave(gens, weights)

    nc.compile()
    return nc


_NC_CACHE = {}


def _get_nc():
    if "nc" not in _NC_CACHE:
        _NC_CACHE["nc"] = build_nc()
    return _NC_CACHE["nc"]


def prepare_in_maps(ix, Wq, Wk, Wv, Wp):
    ix = np.asarray(ix, np.float32)
    Wq = np.asarray(Wq, np.float32)
    Wk = np.asarray(Wk, np.float32)
    Wv = np.asarray(Wv, np.float32)
    Wp = np.asarray(Wp, np.float32)

    xt = np.ascontiguousarray(ix.transpose(0, 2, 1))  # [B, C, T]
    mask = np.where(np.arange(128)[:, None] >= np.arange(128)[None, :], 0.0, NEG).astype(np.float32)

    in_maps = []
    for core in range(NCORES):
        h0, h1 = 2 * core, 2 * core + 1
        in_maps.append({
            "x": xt,
            "wq": _r12(np.concatenate([Wq[h0], Wq[h1]], axis=1) * 32.0),  # carries sqrt(C)
            "wk": _r12(np.concatenate([Wk[h0], Wk[h1]], axis=1)),
            "wv": _r12(np.concatenate([Wv[h0], Wv[h1]], axis=1)),
            "wp": _r12(Wp[D * h0:D * (h1 + 1), :]),
            "mask_nat": mask,
            "mask_t": np.ascontiguousarray(mask.T),
        })
    return in_maps


def kernel(ix, Wq, Wk, Wv, Wp, bp):
    in_maps = prepare_in_maps(ix, Wq, Wk, Wv, Wp)
    bp = np.asarray(bp, np.float32)
    nc = _get_nc()
    res = bass_utils.run_bass_kernel_spmd(nc, in_maps, list(range(NCORES)))
    out = np.zeros((B, T, C), np.float64)
    for r in res.results:
        out += np.asarray(r["y"], np.float32)
    out += bp
    return out.astype(np.float32)
